# revision 22
# baseline (speedup 1.0000x reference)
"""Trainium2 Bass kernel for the convolutional differentiable-LUT-tree layer.

v5: no diag builds, no DRAM round-trips, no indirect DMA.  Per tile of
128 nodes the 6-bit LUT interpolation
  out = sum_{a in 64} sigmoid(w[a]) * prod_i (g_i if bit_i(a) else 1-g_i)
uses the Moebius (polynomial) form over bits 0-3 with 4 accumulators
r = (g4,g5); tables are sigmoid+Moebius-folded on the HOST.  The 60
(term t, r) coefficient FMAs are split across three engines:
  DVE : fused scalar_tensor_tensor FMA chains (ping-pong buffers --
        in-place in1==out STT corrupts on HW)
  PE  : diag(c_tr) matmuls, diag matrices precomputed on host and
        streamed from DRAM (double buffered, DMA'd from the ACT queue);
        PSUM accumulates
  ACT : activation scale ops  tmp = X_t*c_tr ; PE identity-matmul-adds
        tmp into PSUM
ACT drains PSUM (+const-term bias); DVE does the 11 subset products,
the adve merge, and the two bit-4/5 lerp levels.  Tree-layer gathers
are PE one-hot matmuls straight out of h0sb/h1sb in SBUF (one-hot
matrices host-built, accumulated in the spare 256:512 PSUM columns,
ACT-drained into g1sb/g2sb) -- the indirect-DMA/DRAM path was both slow
(7.6us per gather) and racy on HW.
8-way shard over patch rows: core c owns image c's 256 rows.
"""

import numpy as np

B_IMG, C_IN, H, W = 8, 16, 32, 32
C_OUT = 64
KH = KW = 5
SH = SW = 2
PH = PW = 2
INPUT_SIZE = C_IN * KH * KW          # 400
OH = OW = 16
BP = B_IMG * OH * OW                 # 2048
N_CORES = 8
B_LOC = BP // N_CORES                # 256

L0_NODES, L1_NODES, L2_NODES = C_OUT * 36, C_OUT * 6, C_OUT
L0_TILES, L1_TILES = L0_NODES // 128, L1_NODES // 128        # 18, 3
N_TILES = L0_TILES + L1_TILES + 1                            # 22

# term t (4-bit, bit3<->g0, bit2<->g1, bit1<->g2, bit0<->g3) -> moving operand
# 'g', k  : gathered input slot k of the tile
# 'p', j  : product slot j
T_SRC = {
    8: ('g', 0), 4: ('g', 1), 2: ('g', 2), 1: ('g', 3),
    12: ('p', 0), 10: ('p', 1), 9: ('p', 2), 6: ('p', 3), 5: ('p', 4),
    3: ('p', 5), 14: ('p', 6), 13: ('p', 7), 11: ('p', 8), 7: ('p', 9),
    15: ('p', 10),
}
# products: j -> (srcA, srcB); 'g' = gathered slot, 'p' = earlier product
PROD_DEF = [
    (('g', 0), ('g', 1)),   # 0: p01
    (('g', 0), ('g', 2)),   # 1: p02
    (('g', 0), ('g', 3)),   # 2: p03
    (('g', 1), ('g', 2)),   # 3: p12
    (('g', 1), ('g', 3)),   # 4: p13
    (('g', 2), ('g', 3)),   # 5: p23
    (('p', 0), ('g', 2)),   # 6: p012
    (('p', 0), ('g', 3)),   # 7: p013
    (('p', 1), ('g', 3)),   # 8: p023
    (('p', 3), ('g', 3)),   # 9: p123
    (('p', 0), ('p', 5)),   # 10: p0123 = p01*p23
]

# ---- engine split of the 15 nonzero Moebius terms (x4 r each) ----
PE_TS = [15, 14, 13, 11, 7, 12]      # diag matmuls (streamed diags)
ACT_TS = [10, 9, 6]                  # ACT scale + PE identity-add
DVE_TS = [8, 4, 2, 1, 3, 5]          # fused STT chains
N_PE = len(PE_TS)
N_ACT = len(ACT_TS)
PE_PAIRS = [(t, r) for r in range(4) for t in PE_TS]   # DRAM diag order

# L1 gather chunk j sources h0 tiles SRC1[j][0]..SRC1[j][1] inclusive and
# may start once L0 tiles < G1_NEED[j] left h0sb (sTT)
SRC1 = [(0, 6), (5, 12), (11, 17)]
G1_NEED = [7, 13, 18]
OH_BASE = [0]
for _lo, _hi in SRC1:
    OH_BASE.append(OH_BASE[-1] + 6 * (_hi - _lo + 1))
OH_L2 = OH_BASE[-1]
# PE-stream position (run gather group j after PE tile GATHER_AT[j]);
# group 3 is the L2 gather out of h1sb
GATHER_AT = [8, 14, 18, 20]
N_OH = OH_L2 + 6 * L1_TILES                 # one-hot matrix count

USE_STT = True       # fused scalar_tensor_tensor FMA vs mul+add pair
PP_LAST = (len(DVE_TS) - 1) % 2 if USE_STT else 0   # ping-pong slot of final

_CACHE = {}


def _unfold_np(x):
    xp = np.pad(x, ((0, 0), (0, 0), (PH, PH), (PW, PW)))
    ri = (np.arange(OH) * SH)[:, None] + np.arange(KH)[None, :]
    ci = (np.arange(OW) * SW)[:, None] + np.arange(KW)[None, :]
    p = xp[:, :, ri[:, None, :, None], ci[None, :, None, :]]
    p = np.transpose(p, (0, 2, 3, 1, 4, 5))
    return p.reshape(BP, INPUT_SIZE)


def _fold_tables(tbl, n_tiles):
    """sigmoid + Moebius transform over bits 0-3, node-major flat
    [128, n_tiles*64] fp32, col = tile*64 + t*4 + r."""
    t = np.asarray(tbl, np.float64).reshape(-1, 64)
    t = 1.0 / (1.0 + np.exp(-t))
    w = t.reshape(-1, 2, 2, 2, 2, 4)          # a = g0*32+g1*16+g2*8+g3*4+r
    for ax in (1, 2, 3, 4):
        sl1 = tuple(slice(None) if i != ax else 1 for i in range(6))
        sl0 = tuple(slice(None) if i != ax else 0 for i in range(6))
        w[sl1] -= w[sl0]
    c = np.ascontiguousarray(w.reshape(-1, 64), np.float32)
    pad = n_tiles * 128 - c.shape[0]
    if pad:
        c = np.concatenate([c, np.zeros((pad, 64), np.float32)], 0)
    c = c.reshape(n_tiles, 128, 64).transpose(1, 0, 2)
    return np.ascontiguousarray(c.reshape(128, n_tiles * 64))


def _build_diags(coef):
    """coef: [128, 22*64] folded coeffs  ->  [128, 22, 4*N_PE, 128] fp16
    diag(c_{t,r}) per tile for the PE-assigned (t,r) pairs."""
    d = np.zeros((128, N_TILES, 4 * N_PE, 128), np.float16)
    p = np.arange(128)
    for T in range(N_TILES):
        for j, (t, r) in enumerate(PE_PAIRS):
            d[p, T, j, p] = coef[:, T * 64 + t * 4 + r].astype(np.float16)
    return d


def _build_onehots(idx1, idx2):
    """One-hot gather matrices for the tree layers, [128, N_OH, 128] fp16.
    L1 chunk j, slot k, src tile c at j*36+k*6+c:  oh[p, ., q] = 1 iff
    h0 row idx1g[128j+q, k] == (6j+c)*128 + p.  L2 at 108+k*3+c from h1."""
    t_of_l1 = np.repeat(np.arange(C_OUT), 6)
    idx1g = np.asarray(idx1).reshape(L1_NODES, 6) + t_of_l1[:, None] * 36
    idx2g = np.asarray(idx2).reshape(L2_NODES, 6) + np.arange(C_OUT)[:, None] * 6
    oh = np.zeros((128, N_OH, 128), np.float16)
    q = np.arange(128)
    for j in range(L1_TILES):
        lo, hi = SRC1[j]
        nsrc = hi - lo + 1
        for k in range(6):
            rows = idx1g[128 * j:128 * (j + 1), k]
            for ci, c in enumerate(range(lo, hi + 1)):
                sel = rows - c * 128
                m = (sel >= 0) & (sel < 128)
                oh[sel[m], OH_BASE[j] + k * nsrc + ci, q[m]] = 1.0
    q64 = np.arange(64)
    for k in range(6):
        rows = idx2g[:, k]
        for c in range(L1_TILES):
            sel = rows - c * 128
            m = (sel >= 0) & (sel < 128)
            oh[sel[m], OH_L2 + k * 3 + c, q64[m]] = 1.0
    return oh


def _build_program():
    import concourse.bass as bass
    import concourse.mybir as mybir

    f16, f32 = mybir.dt.float16, mybir.dt.float32
    AF = mybir.ActivationFunctionType
    MUL, ADD = mybir.AluOpType.mult, mybir.AluOpType.add

    nc = bass.Bass()

    g0in = nc.declare_dram_parameter("g0", [128, L0_TILES * 6, B_LOC], f16, isOutput=False)
    coefin = nc.declare_dram_parameter("coef", [128, N_TILES * 64], f32, isOutput=False)
    diagin = (nc.declare_dram_parameter("diags", [128, N_TILES, 4 * N_PE, 128], f16, isOutput=False)
              if N_PE else None)
    identin = nc.declare_dram_parameter("ident", [128, 128], f16, isOutput=False)
    ohin = nc.declare_dram_parameter("onehots", [128, N_OH, 128], f16, isOutput=False)
    out = nc.declare_dram_parameter("out", [C_OUT, B_LOC], f32, isOutput=True)

    layer_of = [0] * L0_TILES + [1] * L1_TILES + [2]
    tile_base = [0, L0_TILES, L0_TILES + L1_TILES]

    from contextlib import ExitStack
    es = ExitStack()
    with es:
        sb = lambda *a: es.enter_context(nc.sbuf_tensor(*a))
        sem = lambda n: es.enter_context(nc.semaphore(n))
        g0sb = sb("g0sb", [128, L0_TILES * 6, B_LOC], f16)
        g1sb = sb("g1sb", [128, L1_TILES * 6, B_LOC], f16)
        g2sb = sb("g2sb", [128, 6, B_LOC], f16)
        coef = sb("coefsb", [128, N_TILES * 64], f32)
        diag = sb("diagsb", [128, 2, 4 * N_PE, 128], f16) if N_PE else None
        ident = sb("identsb", [128, 128], f16)
        ohsb = sb("ohsb", [128, N_OH, 128], f16)
        prod = sb("prod", [128, 2, 11, B_LOC], f16)
        adve = sb("adve", [128, 2, 4, 2, B_LOC], f16)   # ping-pong pairs
        atmp = sb("atmp", [128, 2, 4 * N_ACT, B_LOC], f16) if N_ACT else None
        acc45 = sb("acc45", [128, 2, 4, B_LOC], f16)
        tchn = sb("tchn", [128, B_LOC], f16)
        h0sb = sb("h0sb", [128, L0_TILES, B_LOC], f16)
        h1sb = sb("h1sb", [128, L1_TILES, B_LOC], f16)
        h2sb = sb("h2sb", [64, B_LOC], f32)
        ps = es.enter_context(nc.psum_tensor("ps", [128, 8, 512], f32))

        sING = sem("sING")          # ident/coef in (SP ring)
        sOH = sem("sOH")            # one-hots in (ACT ring)
        sG0 = sem("sG0")            # per-tile g0 chunks (16 each)
        sDGD = sem("sDGD")          # per-tile diag DMA (16 each)
        sPR = sem("sPR")            # products done, per tile
        sDV = sem("sDV")            # DVE chains done, per tile
        sSC = sem("sSC")            # ACT scales done, per tile
        sMM = sem("sMM")            # PE done, per tile
        sDR = sem("sDR")            # drains done, per tile
        sTT = sem("sTT")            # lerps done, per tile
        sGM2 = sem("sGM2")          # PE gather slot-groups done (1 per slot)
        sGDs = sem("sGDs")          # ACT gather slot drains (1 per slot)
        sOUT = sem("sOUT")
        block = es.enter_context(nc.Block())

        def gslot(T, k):
            lay = layer_of[T]
            Tl = T - tile_base[lay]
            w = 64 if lay == 2 else 128
            gsbuf = [g0sb, g1sb, g2sb][lay]
            if lay == 2:
                return gsbuf[0:w, k, :]
            return gsbuf[0:w, Tl * 6 + k, :]

        def srcap(T, kind, j, w):
            if kind == 'g':
                return gslot(T, j)
            return prod[0:w, T % 2, j, :]

        def cslice(T, t, r, w=128):
            i = T * 64 + t * 4 + r
            return coef[0:w, i:i + 1]

        def wid(T):
            return 64 if layer_of[T] == 2 else 128

        # ---------------- sync (SP queue): ident/coef/g0 in, out ---------
        @block.sync
        def _(sync):
            sync.dma_start(out=ident[:], in_=identin[:]).then_inc(sING, 16)
            for T in range(2):
                s0, s1 = T * 6, T * 6 + 6
                sync.dma_start(
                    out=g0sb[:, s0:s1, :], in_=g0in[:, s0:s1, :]
                ).then_inc(sG0, 16)
            sync.dma_start(out=coef[:], in_=coefin[:]).then_inc(sING, 16)
            for T in range(2, L0_TILES):
                s0, s1 = T * 6, T * 6 + 6
                sync.dma_start(
                    out=g0sb[:, s0:s1, :], in_=g0in[:, s0:s1, :]
                ).then_inc(sG0, 16)
            sync.wait_ge(sTT, N_TILES)
            sync.dma_start(out=out[:], in_=h2sb[:]).then_inc(sOUT, 16)
            sync.wait_ge(sOUT, 16)

        # ---------------- vector: products, STT chains, lerps ------------
        @block.vector
        def _(dve):
            def products(T):
                w = wid(T)
                lay = layer_of[T]
                if lay == 0:
                    dve.wait_ge(sG0, 16 * (T + 1))
                elif lay == 1:
                    dve.wait_ge(sGDs, 6 * (T - L0_TILES + 1))
                else:
                    dve.wait_ge(sGDs, 24)
                if T >= 2:
                    if N_PE + N_ACT:
                        dve.wait_ge(sMM, T - 1)  # prod/atmp consumers done
                    else:
                        dve.wait_ge(sDR, T - 1)  # adve consumed by drains
                for j, (a, b) in enumerate(PROD_DEF):
                    ins = dve.tensor_mul(
                        prod[0:w, T % 2, j, :],
                        srcap(T, a[0], a[1], w),
                        srcap(T, b[0], b[1], w))
                ins.then_inc(sPR, 1)

            def chains(T):
                w = wid(T)
                ins = None
                for r in range(4):
                    t0 = DVE_TS[0]
                    k0, j0_ = T_SRC[t0]
                    dve.tensor_scalar_mul(
                        adve[0:w, T % 2, r, 0, :],
                        srcap(T, k0, j0_, w), cslice(T, t0, r, w))
                    pp = 0
                    for t in DVE_TS[1:]:
                        kk, jj = T_SRC[t]
                        if USE_STT:
                            ins = dve.scalar_tensor_tensor(
                                adve[0:w, T % 2, r, 1 - pp, :],
                                srcap(T, kk, jj, w), cslice(T, t, r, w),
                                adve[0:w, T % 2, r, pp, :], MUL, ADD)
                            pp = 1 - pp
                        else:
                            dve.tensor_scalar_mul(
                                tchn[0:w, :],
                                srcap(T, kk, jj, w), cslice(T, t, r, w))
                            ins = dve.tensor_add(
                                adve[0:w, T % 2, r, 0, :],
                                adve[0:w, T % 2, r, 0, :], tchn[0:w, :])
                ins.then_inc(sDV, 1)

            def lerps(T):
                w = wid(T)
                lay = layer_of[T]
                Tl = T - tile_base[lay]
                dve.wait_ge(sDR, T + 1)
                a = acc45[0:w, T % 2]
                lo, hi = a[:, 0:2, :], a[:, 2:4, :]
                g4 = gslot(T, 4).unsqueeze(1).broadcast_to([w, 2, B_LOC])
                dve.tensor_sub(hi, hi, lo)
                dve.tensor_mul(hi, hi, g4)
                dve.tensor_add(lo, lo, hi)
                l1, h1_ = a[:, 0, :], a[:, 1, :]
                g5 = gslot(T, 5)
                dve.tensor_sub(h1_, h1_, l1)
                dve.tensor_mul(h1_, h1_, g5)
                if lay == 0:
                    dst = h0sb[:, Tl, :]
                elif lay == 1:
                    dst = h1sb[:, Tl, :]
                else:
                    dst = h2sb[:]
                dve.tensor_add(dst, l1, h1_).then_inc(sTT, 1)

            products(0)
            dve.wait_ge(sING, 32)
            if DVE_TS:
                chains(0)
            for T in range(1, 21):
                if T >= 2:
                    lerps(T - 2)
                products(T)
                if DVE_TS:
                    chains(T)
            # tail: L2 tile needs h1 complete before its products
            lerps(19)
            lerps(20)
            products(21)
            if DVE_TS:
                chains(21)
            lerps(21)

        # ---------------- scalar: drains, scales, gather drains, DMA -----
        @block.scalar
        def _(act):
            def scales(T):
                w = wid(T)
                act.wait_ge(sPR, T + 1)
                if T >= 2:
                    act.wait_ge(sMM, T - 1)      # atmp slot free
                ins = None
                for i, t in enumerate(ACT_TS):
                    kk, jj = T_SRC[t]
                    for r in range(4):
                        ins = act.activation(
                            atmp[0:w, T % 2, i * 4 + r, :],
                            srcap(T, kk, jj, w),
                            AF.Identity, bias=0.0, scale=cslice(T, t, r, w))
                ins.then_inc(sSC, 1)

            def drains(T):
                w = wid(T)
                if N_PE + N_ACT:
                    act.wait_ge(sMM, T + 1)
                else:
                    act.wait_ge(sDV, T + 1)
                if T >= 2:
                    act.wait_ge(sTT, T - 1)      # acc45 slot free
                ins = None
                for r in range(4):
                    src_ap = (ps[0:w, (T % 2) * 4 + r, 0:B_LOC]
                              if N_PE + N_ACT else adve[0:w, T % 2, r, PP_LAST, :])
                    ins = act.activation(
                        acc45[0:w, T % 2, r, :], src_ap,
                        AF.Identity, bias=cslice(T, 0, r, w), scale=1.0)
                ins.then_inc(sDR, 1)

            def gdrain(j):
                # drain gather slots (borrowed PSUM set 1) into g1sb/g2sb
                w = 64 if j == 3 else 128
                for k in range(6):
                    act.wait_ge(sGM2, 6 * j + k + 1)
                    dst = (g2sb[0:w, k, :] if j == 3
                           else g1sb[0:w, j * 6 + k, :])
                    act.activation(
                        dst, ps[0:w, 4 + (k % 4), 0:B_LOC],
                        AF.Identity, bias=0.0, scale=1.0
                    ).then_inc(sGDs, 1)

            # diag + one-hot DMAs ride the ACT queue
            for T in range(2):
                if N_PE:
                    act.dma_start(
                        out=diag[:, T % 2, :, :], in_=diagin[:, T, :, :]
                    ).then_inc(sDGD, 16)
            act.wait_ge(sING, 32)
            if ACT_TS:
                scales(0)
            for T in range(1, N_TILES):
                drains(T - 1)
                for j in range(4):
                    if T == GATHER_AT[j] + 1:
                        gdrain(j)
                if ACT_TS:
                    scales(T)
                if T + 1 < N_TILES and N_PE:
                    # slot (T+1)%2 is free: drains(T-1) waited sMM >= T
                    act.dma_start(
                        out=diag[:, (T + 1) % 2, :, :], in_=diagin[:, T + 1, :, :]
                    ).then_inc(sDGD, 16)
                if 3 <= T <= 5:
                    lo = [0, OH_BASE[1], OH_BASE[2]][T - 3]
                    hi = [OH_BASE[1], OH_BASE[2], N_OH][T - 3]
                    act.dma_start(
                        out=ohsb[:, lo:hi, :], in_=ohin[:, lo:hi, :]
                    ).then_inc(sOH, 16)
            drains(N_TILES - 1)

        # ---------------- tensor: diag matmuls, id-adds, gathers ---------
        @block.tensor
        def _(pe):
            n_per_bank = N_PE + N_ACT + (1 if DVE_TS else 0)

            def gather(j):
                # one-hot gather group j: L1 chunk j from h0sb, or (j==3)
                # the L2 gather from h1sb; PSUM cols 256:512 of banks 0..5
                pe.wait_ge(sOH, 16 * (j + 1) if j < 3 else 48)
                X = GATHER_AT[j]
                pe.wait_ge(sDR, X)          # borrowed set 1 drained
                w = 64 if j == 3 else 128
                if j < 3:
                    pe.wait_ge(sTT, G1_NEED[j])
                    lo, hi = SRC1[j]
                    nsrc = hi - lo + 1
                    srcs = [h0sb[:, c, :] for c in range(lo, hi + 1)]
                    base = OH_BASE[j]
                else:
                    pe.wait_ge(sTT, L0_TILES + L1_TILES)
                    nsrc = L1_TILES
                    srcs = [h1sb[:, c, :] for c in range(L1_TILES)]
                    base = OH_L2
                for k in range(6):
                    if k >= 4:
                        pe.wait_ge(sGDs, 6 * j + k - 3)   # bank drained
                    for ci in range(nsrc):
                        ins = pe.matmul(
                            ps[0:w, 4 + (k % 4), 0:B_LOC],
                            ohsb[:, base + k * nsrc + ci, 0:w],
                            srcs[ci],
                            start=(ci == 0), stop=(ci == nsrc - 1))
                    ins.then_inc(sGM2, 1)

            def tile(T):
                w = wid(T)
                done = [0, 0, 0, 0]     # matmuls issued per bank r
                pe.wait_ge(sPR, T + 1)
                if N_PE:
                    pe.wait_ge(sDGD, 16 * (T + 1))
                if T >= 2:
                    pe.wait_ge(sDR, T - 1)       # PSUM set free
                for jj in range(4):
                    if T == GATHER_AT[jj] + 1:
                        pe.wait_ge(sGDs, 6 * (jj + 1))

                def mm(r, lhsT, rhs):
                    bank = (T % 2) * 4 + r
                    ins = pe.matmul(
                        ps[0:w, bank, 0:B_LOC], lhsT, rhs,
                        start=(done[r] == 0), stop=(done[r] == n_per_bank - 1))
                    done[r] += 1
                    return ins

                ins = None
                for r in range(4):
                    for i, t in enumerate(PE_TS):
                        kk, jj = T_SRC[t]
                        ins = mm(r, diag[0:w, T % 2, r * N_PE + i, 0:w],
                                 srcap(T, kk, jj, w))
                if N_ACT:
                    pe.wait_ge(sSC, T + 1)
                    for i in range(N_ACT):
                        for r in range(4):
                            ins = mm(r, ident[0:w, 0:w],
                                     atmp[0:w, T % 2, i * 4 + r, :])
                if DVE_TS:
                    pe.wait_ge(sDV, T + 1)
                    for r in range(4):
                        ins = mm(r, ident[0:w, 0:w],
                                 adve[0:w, T % 2, r, PP_LAST, :])
                ins.then_inc(sMM, 1)

            for T in range(N_TILES):
                tile(T)
                for j in range(4):
                    if T == GATHER_AT[j]:
                        gather(j)

    return nc


def _get_program():
    if "nc" not in _CACHE:
        _CACHE["nc"] = _build_program()
    return _CACHE["nc"]


def prepare_inputs(x, idx0, table0, idx1, table1, idx2, table2):
    p = _unfold_np(np.asarray(x, np.float32))             # [2048, 400]
    idx0f = np.asarray(idx0).reshape(L0_NODES, 6)
    j0 = np.empty(L0_TILES * 6 * 128, np.int64)
    for tile in range(L0_TILES):
        for k in range(6):
            slot = tile * 6 + k
            j0[slot * 128:(slot + 1) * 128] = idx0f[tile * 128:(tile + 1) * 128, k]
    c0 = _fold_tables(table0, L0_TILES)
    c1 = _fold_tables(table1, L1_TILES)
    c2 = _fold_tables(table2, 1)
    coef = np.concatenate([c0, c1, c2], axis=1)           # [128, 22*64]
    diags = _build_diags(coef) if N_PE else None
    oh = _build_onehots(idx1, idx2)
    ident = np.eye(128, dtype=np.float16)
    in_maps = []
    for c in range(N_CORES):
        pcT = p[c * B_LOC:(c + 1) * B_LOC, :].T.astype(np.float16)  # [400, 256]
        g0 = np.ascontiguousarray(
            pcT[j0].reshape(L0_TILES * 6, 128, B_LOC).transpose(1, 0, 2)
        )
        m = {"g0": g0, "coef": coef, "ident": ident, "onehots": oh}
        if N_PE:
            m["diags"] = diags
        in_maps.append(m)
    return in_maps


def assemble_output(per_core_out):
    h2 = np.stack(per_core_out, 0)                        # [8, 64, 256]
    return np.ascontiguousarray(h2.reshape(B_IMG, C_OUT, OH, OW).astype(np.float32))


def kernel(x, idx0, table0, idx1, table1, idx2, table2):
    from concourse.bass_utils import run_bass_kernel_spmd

    nc = _get_program()
    in_maps = prepare_inputs(x, idx0, table0, idx1, table1, idx2, table2)
    res = run_bass_kernel_spmd(nc, in_maps, list(range(N_CORES)))
    outs = [np.asarray(res.results[c]["out"], np.float32) for c in range(N_CORES)]
    return assemble_output(outs)


# revision 23
# speedup vs baseline: 1.0407x; 1.0407x over previous
"""Trainium2 Bass kernel for the convolutional differentiable-LUT-tree layer.

v5: no diag builds, no DRAM round-trips, no indirect DMA.  Per tile of
128 nodes the 6-bit LUT interpolation
  out = sum_{a in 64} sigmoid(w[a]) * prod_i (g_i if bit_i(a) else 1-g_i)
uses the Moebius (polynomial) form over bits 0-3 with 4 accumulators
r = (g4,g5); tables are sigmoid+Moebius-folded on the HOST.  The 60
(term t, r) coefficient FMAs are split across three engines:
  DVE : fused scalar_tensor_tensor FMA chains (ping-pong buffers --
        in-place in1==out STT corrupts on HW)
  PE  : diag(c_tr) matmuls, diag matrices precomputed on host and
        streamed from DRAM (double buffered, DMA'd from the ACT queue);
        PSUM accumulates
  ACT : activation scale ops  tmp = X_t*c_tr ; PE identity-matmul-adds
        tmp into PSUM
ACT drains PSUM (+const-term bias); DVE does the 11 subset products,
the adve merge, and the two bit-4/5 lerp levels.  Tree-layer gathers
are PE one-hot matmuls straight out of h0sb/h1sb in SBUF (one-hot
matrices host-built, accumulated in the spare 256:512 PSUM columns,
ACT-drained into g1sb/g2sb) -- the indirect-DMA/DRAM path was both slow
(7.6us per gather) and racy on HW.
8-way shard over patch rows: core c owns image c's 256 rows.
"""

import numpy as np

B_IMG, C_IN, H, W = 8, 16, 32, 32
C_OUT = 64
KH = KW = 5
SH = SW = 2
PH = PW = 2
INPUT_SIZE = C_IN * KH * KW          # 400
OH = OW = 16
BP = B_IMG * OH * OW                 # 2048
N_CORES = 8
B_LOC = BP // N_CORES                # 256

L0_NODES, L1_NODES, L2_NODES = C_OUT * 36, C_OUT * 6, C_OUT
L0_TILES, L1_TILES = L0_NODES // 128, L1_NODES // 128        # 18, 3
N_TILES = L0_TILES + L1_TILES + 1                            # 22

# term t (4-bit, bit3<->g0, bit2<->g1, bit1<->g2, bit0<->g3) -> moving operand
# 'g', k  : gathered input slot k of the tile
# 'p', j  : product slot j
T_SRC = {
    8: ('g', 0), 4: ('g', 1), 2: ('g', 2), 1: ('g', 3),
    12: ('p', 0), 10: ('p', 1), 9: ('p', 2), 6: ('p', 3), 5: ('p', 4),
    3: ('p', 5), 14: ('p', 6), 13: ('p', 7), 11: ('p', 8), 7: ('p', 9),
    15: ('p', 10),
}
# products: j -> (srcA, srcB); 'g' = gathered slot, 'p' = earlier product
PROD_DEF = [
    (('g', 0), ('g', 1)),   # 0: p01
    (('g', 0), ('g', 2)),   # 1: p02
    (('g', 0), ('g', 3)),   # 2: p03
    (('g', 1), ('g', 2)),   # 3: p12
    (('g', 1), ('g', 3)),   # 4: p13
    (('g', 2), ('g', 3)),   # 5: p23
    (('p', 0), ('g', 2)),   # 6: p012
    (('p', 0), ('g', 3)),   # 7: p013
    (('p', 1), ('g', 3)),   # 8: p023
    (('p', 3), ('g', 3)),   # 9: p123
    (('p', 0), ('p', 5)),   # 10: p0123 = p01*p23
]

# ---- engine split of the 15 nonzero Moebius terms (x4 r each) ----
PE_TS = [15, 14, 13, 11, 7, 12]      # diag matmuls (streamed diags)
ACT_TS = [10, 9, 6]                  # ACT scale + PE identity-add
DVE_TS = [8, 4, 2, 1, 3, 5]          # fused STT chains
N_PE = len(PE_TS)
N_ACT = len(ACT_TS)
PE_PAIRS = [(t, r) for r in range(4) for t in PE_TS]   # DRAM diag order

# L1 gather chunk j sources h0 tiles SRC1[j][0]..SRC1[j][1] inclusive and
# may start once L0 tiles < G1_NEED[j] left h0sb (sTT)
SRC1 = [(0, 6), (5, 12), (11, 17)]
G1_NEED = [7, 13, 18]
OH_BASE = [0]
for _lo, _hi in SRC1:
    OH_BASE.append(OH_BASE[-1] + 6 * (_hi - _lo + 1))
OH_L2 = OH_BASE[-1]
# PE-stream position (run gather group j after PE tile GATHER_AT[j]);
# group 3 is the L2 gather out of h1sb
GATHER_AT = [8, 14, 18, 20]
N_OH = OH_L2 + 6 * L1_TILES                 # one-hot matrix count

USE_STT = True       # fused scalar_tensor_tensor FMA vs mul+add pair
PP_LAST = (len(DVE_TS) - 1) % 2 if USE_STT else 0   # ping-pong slot of final

_CACHE = {}


def _unfold_np(x):
    xp = np.pad(x, ((0, 0), (0, 0), (PH, PH), (PW, PW)))
    ri = (np.arange(OH) * SH)[:, None] + np.arange(KH)[None, :]
    ci = (np.arange(OW) * SW)[:, None] + np.arange(KW)[None, :]
    p = xp[:, :, ri[:, None, :, None], ci[None, :, None, :]]
    p = np.transpose(p, (0, 2, 3, 1, 4, 5))
    return p.reshape(BP, INPUT_SIZE)


def _fold_tables(tbl, n_tiles):
    """sigmoid + Moebius transform over bits 0-3, node-major flat
    [128, n_tiles*64] fp32, col = tile*64 + t*4 + r."""
    t = np.asarray(tbl, np.float64).reshape(-1, 64)
    t = 1.0 / (1.0 + np.exp(-t))
    w = t.reshape(-1, 2, 2, 2, 2, 4)          # a = g0*32+g1*16+g2*8+g3*4+r
    for ax in (1, 2, 3, 4):
        sl1 = tuple(slice(None) if i != ax else 1 for i in range(6))
        sl0 = tuple(slice(None) if i != ax else 0 for i in range(6))
        w[sl1] -= w[sl0]
    c = np.ascontiguousarray(w.reshape(-1, 64), np.float32)
    pad = n_tiles * 128 - c.shape[0]
    if pad:
        c = np.concatenate([c, np.zeros((pad, 64), np.float32)], 0)
    c = c.reshape(n_tiles, 128, 64).transpose(1, 0, 2)
    return np.ascontiguousarray(c.reshape(128, n_tiles * 64))


def _build_diags(coef):
    """coef: [128, 22*64] folded coeffs  ->  [128, 22, 4*N_PE, 128] fp16
    diag(c_{t,r}) per tile for the PE-assigned (t,r) pairs."""
    d = np.zeros((128, N_TILES, 4 * N_PE, 128), np.float16)
    p = np.arange(128)
    for T in range(N_TILES):
        for j, (t, r) in enumerate(PE_PAIRS):
            d[p, T, j, p] = coef[:, T * 64 + t * 4 + r].astype(np.float16)
    return d


def _build_onehots(idx1, idx2):
    """One-hot gather matrices for the tree layers, [128, N_OH, 128] fp16.
    L1 chunk j, slot k, src tile c at j*36+k*6+c:  oh[p, ., q] = 1 iff
    h0 row idx1g[128j+q, k] == (6j+c)*128 + p.  L2 at 108+k*3+c from h1."""
    t_of_l1 = np.repeat(np.arange(C_OUT), 6)
    idx1g = np.asarray(idx1).reshape(L1_NODES, 6) + t_of_l1[:, None] * 36
    idx2g = np.asarray(idx2).reshape(L2_NODES, 6) + np.arange(C_OUT)[:, None] * 6
    oh = np.zeros((128, N_OH, 128), np.float16)
    q = np.arange(128)
    for j in range(L1_TILES):
        lo, hi = SRC1[j]
        nsrc = hi - lo + 1
        for k in range(6):
            rows = idx1g[128 * j:128 * (j + 1), k]
            for ci, c in enumerate(range(lo, hi + 1)):
                sel = rows - c * 128
                m = (sel >= 0) & (sel < 128)
                oh[sel[m], OH_BASE[j] + k * nsrc + ci, q[m]] = 1.0
    q64 = np.arange(64)
    for k in range(6):
        rows = idx2g[:, k]
        for c in range(L1_TILES):
            sel = rows - c * 128
            m = (sel >= 0) & (sel < 128)
            oh[sel[m], OH_L2 + k * 3 + c, q64[m]] = 1.0
    return oh


def _build_program():
    import concourse.bass as bass
    import concourse.mybir as mybir

    f16, f32 = mybir.dt.float16, mybir.dt.float32
    AF = mybir.ActivationFunctionType
    MUL, ADD = mybir.AluOpType.mult, mybir.AluOpType.add

    nc = bass.Bass()

    g0in = nc.declare_dram_parameter("g0", [128, L0_TILES * 6, B_LOC], f16, isOutput=False)
    coefin = nc.declare_dram_parameter("coef", [128, N_TILES * 64], f32, isOutput=False)
    diagin = (nc.declare_dram_parameter("diags", [128, N_TILES, 4 * N_PE, 128], f16, isOutput=False)
              if N_PE else None)
    identin = nc.declare_dram_parameter("ident", [128, 128], f16, isOutput=False)
    ohin = nc.declare_dram_parameter("onehots", [128, N_OH, 128], f16, isOutput=False)
    out = nc.declare_dram_parameter("out", [C_OUT, B_LOC], f32, isOutput=True)

    layer_of = [0] * L0_TILES + [1] * L1_TILES + [2]
    tile_base = [0, L0_TILES, L0_TILES + L1_TILES]

    from contextlib import ExitStack
    es = ExitStack()
    with es:
        sb = lambda *a: es.enter_context(nc.sbuf_tensor(*a))
        sem = lambda n: es.enter_context(nc.semaphore(n))
        g0sb = sb("g0sb", [128, L0_TILES * 6, B_LOC], f16)
        g1sb = sb("g1sb", [128, L1_TILES * 6, B_LOC], f16)
        g2sb = sb("g2sb", [128, 6, B_LOC], f16)
        coef = sb("coefsb", [128, N_TILES * 64], f32)
        diag = sb("diagsb", [128, 2, 4 * N_PE, 128], f16) if N_PE else None
        ident = sb("identsb", [128, 128], f16)
        ohsb = sb("ohsb", [128, N_OH, 128], f16)
        prod = sb("prod", [128, 2, 11, B_LOC], f16)
        adve = sb("adve", [128, 2, 4, 2, B_LOC], f16)   # ping-pong pairs
        atmp = sb("atmp", [128, 2, 4 * N_ACT, B_LOC], f16) if N_ACT else None
        acc45 = sb("acc45", [128, 2, 4, B_LOC], f16)
        tchn = sb("tchn", [128, B_LOC], f16)
        h0sb = sb("h0sb", [128, L0_TILES, B_LOC], f16)
        h1sb = sb("h1sb", [128, L1_TILES, B_LOC], f16)
        h2sb = sb("h2sb", [64, B_LOC], f32)
        ps = es.enter_context(nc.psum_tensor("ps", [128, 8, 512], f32))

        sING = sem("sING")          # ident/coef in (SP ring)
        sOH = sem("sOH")            # one-hots in (ACT ring)
        sG0 = sem("sG0")            # per-tile g0 chunks (16 each)
        sDGD = sem("sDGD")          # per-tile diag DMA (16 each)
        sPR = sem("sPR")            # products done, per tile
        sDV = sem("sDV")            # DVE chains done, per tile
        sSC = sem("sSC")            # ACT scales done, per tile
        sMM = sem("sMM")            # PE done, per tile
        sDR = sem("sDR")            # drains done, per tile
        sTT = sem("sTT")            # lerps done, per tile
        sGM2 = sem("sGM2")          # PE gather slot-groups done (1 per slot)
        sGDs = sem("sGDs")          # ACT gather slot drains (1 per slot)
        sOUT = sem("sOUT")
        block = es.enter_context(nc.Block())

        def gslot(T, k):
            lay = layer_of[T]
            Tl = T - tile_base[lay]
            w = 64 if lay == 2 else 128
            gsbuf = [g0sb, g1sb, g2sb][lay]
            if lay == 2:
                return gsbuf[0:w, k, :]
            return gsbuf[0:w, Tl * 6 + k, :]

        def srcap(T, kind, j, w):
            if kind == 'g':
                return gslot(T, j)
            return prod[0:w, T % 2, j, :]

        def cslice(T, t, r, w=128):
            i = T * 64 + t * 4 + r
            return coef[0:w, i:i + 1]

        def wid(T):
            return 64 if layer_of[T] == 2 else 128

        # ---------------- sync (SP queue): ident/coef/g0 in, out ---------
        @block.sync
        def _(sync):
            sync.dma_start(out=ident[:], in_=identin[:]).then_inc(sING, 16)
            for T in range(2):
                s0, s1 = T * 6, T * 6 + 6
                sync.dma_start(
                    out=g0sb[:, s0:s1, :], in_=g0in[:, s0:s1, :]
                ).then_inc(sG0, 16)
            sync.dma_start(out=coef[:], in_=coefin[:]).then_inc(sING, 16)
            for T in range(2, L0_TILES):
                s0, s1 = T * 6, T * 6 + 6
                sync.dma_start(
                    out=g0sb[:, s0:s1, :], in_=g0in[:, s0:s1, :]
                ).then_inc(sG0, 16)
            sync.wait_ge(sTT, N_TILES)
            sync.dma_start(out=out[:], in_=h2sb[:]).then_inc(sOUT, 16)
            sync.wait_ge(sOUT, 16)

        # ---------------- vector: products, STT chains, lerps ------------
        @block.vector
        def _(dve):
            def products(T):
                w = wid(T)
                lay = layer_of[T]
                if lay == 0:
                    dve.wait_ge(sG0, 16 * (T + 1))
                elif lay == 1:
                    dve.wait_ge(sGDs, 6 * (T - L0_TILES + 1))
                else:
                    dve.wait_ge(sGDs, 24)
                if T >= 2:
                    if N_PE + N_ACT:
                        dve.wait_ge(sMM, T - 1)  # prod/atmp consumers done
                    else:
                        dve.wait_ge(sDR, T - 1)  # adve consumed by drains
                for j, (a, b) in enumerate(PROD_DEF):
                    ins = dve.tensor_mul(
                        prod[0:w, T % 2, j, :],
                        srcap(T, a[0], a[1], w),
                        srcap(T, b[0], b[1], w))
                ins.then_inc(sPR, 1)

            def chains(T):
                w = wid(T)
                ins = None
                for r in range(4):
                    t0 = DVE_TS[0]
                    k0, j0_ = T_SRC[t0]
                    dve.tensor_scalar_mul(
                        adve[0:w, T % 2, r, 0, :],
                        srcap(T, k0, j0_, w), cslice(T, t0, r, w))
                    pp = 0
                    for t in DVE_TS[1:]:
                        kk, jj = T_SRC[t]
                        if USE_STT:
                            ins = dve.scalar_tensor_tensor(
                                adve[0:w, T % 2, r, 1 - pp, :],
                                srcap(T, kk, jj, w), cslice(T, t, r, w),
                                adve[0:w, T % 2, r, pp, :], MUL, ADD)
                            pp = 1 - pp
                        else:
                            dve.tensor_scalar_mul(
                                tchn[0:w, :],
                                srcap(T, kk, jj, w), cslice(T, t, r, w))
                            ins = dve.tensor_add(
                                adve[0:w, T % 2, r, 0, :],
                                adve[0:w, T % 2, r, 0, :], tchn[0:w, :])
                ins.then_inc(sDV, 1)

            def lerps(T):
                w = wid(T)
                lay = layer_of[T]
                Tl = T - tile_base[lay]
                dve.wait_ge(sDR, T + 1)
                a = acc45[0:w, T % 2]
                if DVE_TS and (N_PE + N_ACT):
                    dve.tensor_add(a[:, 0:4, :], a[:, 0:4, :],
                                   adve[0:w, T % 2, 0:4, PP_LAST, :])
                lo, hi = a[:, 0:2, :], a[:, 2:4, :]
                g4 = gslot(T, 4).unsqueeze(1).broadcast_to([w, 2, B_LOC])
                dve.tensor_sub(hi, hi, lo)
                dve.tensor_mul(hi, hi, g4)
                dve.tensor_add(lo, lo, hi)
                l1, h1_ = a[:, 0, :], a[:, 1, :]
                g5 = gslot(T, 5)
                dve.tensor_sub(h1_, h1_, l1)
                dve.tensor_mul(h1_, h1_, g5)
                if lay == 0:
                    dst = h0sb[:, Tl, :]
                elif lay == 1:
                    dst = h1sb[:, Tl, :]
                else:
                    dst = h2sb[:]
                dve.tensor_add(dst, l1, h1_).then_inc(sTT, 1)

            products(0)
            dve.wait_ge(sING, 32)
            if DVE_TS:
                chains(0)
            for T in range(1, 21):
                if T >= 2:
                    lerps(T - 2)
                products(T)
                if DVE_TS:
                    chains(T)
            # tail: L2 tile needs h1 complete before its products
            lerps(19)
            lerps(20)
            products(21)
            if DVE_TS:
                chains(21)
            lerps(21)

        # ---------------- scalar: drains, scales, gather drains, DMA -----
        @block.scalar
        def _(act):
            def scales(T):
                w = wid(T)
                act.wait_ge(sPR, T + 1)
                if T >= 2:
                    act.wait_ge(sMM, T - 1)      # atmp slot free
                ins = None
                for i, t in enumerate(ACT_TS):
                    kk, jj = T_SRC[t]
                    for r in range(4):
                        ins = act.activation(
                            atmp[0:w, T % 2, i * 4 + r, :],
                            srcap(T, kk, jj, w),
                            AF.Identity, bias=0.0, scale=cslice(T, t, r, w))
                ins.then_inc(sSC, 1)

            def drains(T):
                w = wid(T)
                if N_PE + N_ACT:
                    act.wait_ge(sMM, T + 1)
                else:
                    act.wait_ge(sDV, T + 1)
                if T >= 2:
                    act.wait_ge(sTT, T - 1)      # acc45 slot free
                ins = None
                for r in range(4):
                    src_ap = (ps[0:w, (T % 2) * 4 + r, 0:B_LOC]
                              if N_PE + N_ACT else adve[0:w, T % 2, r, PP_LAST, :])
                    ins = act.activation(
                        acc45[0:w, T % 2, r, :], src_ap,
                        AF.Identity, bias=cslice(T, 0, r, w), scale=1.0)
                ins.then_inc(sDR, 1)

            def gdrain(j):
                # drain gather slots (borrowed PSUM set 1) into g1sb/g2sb
                w = 64 if j == 3 else 128
                for k in range(6):
                    act.wait_ge(sGM2, 6 * j + k + 1)
                    dst = (g2sb[0:w, k, :] if j == 3
                           else g1sb[0:w, j * 6 + k, :])
                    act.activation(
                        dst, ps[0:w, 4 + (k % 4), 0:B_LOC],
                        AF.Identity, bias=0.0, scale=1.0
                    ).then_inc(sGDs, 1)

            # diag + one-hot DMAs ride the ACT queue
            for T in range(2):
                if N_PE:
                    act.dma_start(
                        out=diag[:, T % 2, :, :], in_=diagin[:, T, :, :]
                    ).then_inc(sDGD, 16)
            act.wait_ge(sING, 32)
            if ACT_TS:
                scales(0)
            for T in range(1, N_TILES):
                drains(T - 1)
                for j in range(4):
                    if T == GATHER_AT[j] + 1:
                        gdrain(j)
                if ACT_TS:
                    scales(T)
                if T + 1 < N_TILES and N_PE:
                    # slot (T+1)%2 is free: drains(T-1) waited sMM >= T
                    act.dma_start(
                        out=diag[:, (T + 1) % 2, :, :], in_=diagin[:, T + 1, :, :]
                    ).then_inc(sDGD, 16)
                if 3 <= T <= 5:
                    lo = [0, OH_BASE[1], OH_BASE[2]][T - 3]
                    hi = [OH_BASE[1], OH_BASE[2], N_OH][T - 3]
                    act.dma_start(
                        out=ohsb[:, lo:hi, :], in_=ohin[:, lo:hi, :]
                    ).then_inc(sOH, 16)
            drains(N_TILES - 1)

        # ---------------- tensor: diag matmuls, id-adds, gathers ---------
        @block.tensor
        def _(pe):
            n_per_bank = N_PE + N_ACT

            def gather(j):
                # one-hot gather group j: L1 chunk j from h0sb, or (j==3)
                # the L2 gather from h1sb; PSUM cols 256:512 of banks 0..5
                pe.wait_ge(sOH, 16 * (j + 1) if j < 3 else 48)
                X = GATHER_AT[j]
                pe.wait_ge(sDR, X)          # borrowed set 1 drained
                w = 64 if j == 3 else 128
                if j < 3:
                    pe.wait_ge(sTT, G1_NEED[j])
                    lo, hi = SRC1[j]
                    nsrc = hi - lo + 1
                    srcs = [h0sb[:, c, :] for c in range(lo, hi + 1)]
                    base = OH_BASE[j]
                else:
                    pe.wait_ge(sTT, L0_TILES + L1_TILES)
                    nsrc = L1_TILES
                    srcs = [h1sb[:, c, :] for c in range(L1_TILES)]
                    base = OH_L2
                for k in range(6):
                    if k >= 4:
                        pe.wait_ge(sGDs, 6 * j + k - 3)   # bank drained
                    for ci in range(nsrc):
                        ins = pe.matmul(
                            ps[0:w, 4 + (k % 4), 0:B_LOC],
                            ohsb[:, base + k * nsrc + ci, 0:w],
                            srcs[ci],
                            start=(ci == 0), stop=(ci == nsrc - 1))
                    ins.then_inc(sGM2, 1)

            def tile(T):
                w = wid(T)
                done = [0, 0, 0, 0]     # matmuls issued per bank r
                pe.wait_ge(sPR, T + 1)
                if N_PE:
                    pe.wait_ge(sDGD, 16 * (T + 1))
                if T >= 2:
                    pe.wait_ge(sDR, T - 1)       # PSUM set free
                for jj in range(4):
                    if T == GATHER_AT[jj] + 1:
                        pe.wait_ge(sGDs, 6 * (jj + 1))

                def mm(r, lhsT, rhs):
                    bank = (T % 2) * 4 + r
                    ins = pe.matmul(
                        ps[0:w, bank, 0:B_LOC], lhsT, rhs,
                        start=(done[r] == 0), stop=(done[r] == n_per_bank - 1))
                    done[r] += 1
                    return ins

                ins = None
                for r in range(4):
                    for i, t in enumerate(PE_TS):
                        kk, jj = T_SRC[t]
                        ins = mm(r, diag[0:w, T % 2, r * N_PE + i, 0:w],
                                 srcap(T, kk, jj, w))
                if N_ACT:
                    pe.wait_ge(sSC, T + 1)
                    for i in range(N_ACT):
                        for r in range(4):
                            ins = mm(r, ident[0:w, 0:w],
                                     atmp[0:w, T % 2, i * 4 + r, :])
                ins.then_inc(sMM, 1)

            for T in range(N_TILES):
                tile(T)
                for j in range(4):
                    if T == GATHER_AT[j]:
                        gather(j)

    return nc


def _get_program():
    if "nc" not in _CACHE:
        _CACHE["nc"] = _build_program()
    return _CACHE["nc"]


def prepare_inputs(x, idx0, table0, idx1, table1, idx2, table2):
    p = _unfold_np(np.asarray(x, np.float32))             # [2048, 400]
    idx0f = np.asarray(idx0).reshape(L0_NODES, 6)
    j0 = np.empty(L0_TILES * 6 * 128, np.int64)
    for tile in range(L0_TILES):
        for k in range(6):
            slot = tile * 6 + k
            j0[slot * 128:(slot + 1) * 128] = idx0f[tile * 128:(tile + 1) * 128, k]
    c0 = _fold_tables(table0, L0_TILES)
    c1 = _fold_tables(table1, L1_TILES)
    c2 = _fold_tables(table2, 1)
    coef = np.concatenate([c0, c1, c2], axis=1)           # [128, 22*64]
    diags = _build_diags(coef) if N_PE else None
    oh = _build_onehots(idx1, idx2)
    ident = np.eye(128, dtype=np.float16)
    in_maps = []
    for c in range(N_CORES):
        pcT = p[c * B_LOC:(c + 1) * B_LOC, :].T.astype(np.float16)  # [400, 256]
        g0 = np.ascontiguousarray(
            pcT[j0].reshape(L0_TILES * 6, 128, B_LOC).transpose(1, 0, 2)
        )
        m = {"g0": g0, "coef": coef, "ident": ident, "onehots": oh}
        if N_PE:
            m["diags"] = diags
        in_maps.append(m)
    return in_maps


def assemble_output(per_core_out):
    h2 = np.stack(per_core_out, 0)                        # [8, 64, 256]
    return np.ascontiguousarray(h2.reshape(B_IMG, C_OUT, OH, OW).astype(np.float32))


def kernel(x, idx0, table0, idx1, table1, idx2, table2):
    from concourse.bass_utils import run_bass_kernel_spmd

    nc = _get_program()
    in_maps = prepare_inputs(x, idx0, table0, idx1, table1, idx2, table2)
    res = run_bass_kernel_spmd(nc, in_maps, list(range(N_CORES)))
    outs = [np.asarray(res.results[c]["out"], np.float32) for c in range(N_CORES)]
    return assemble_output(outs)
